# revision 72
# baseline (speedup 1.0000x reference)
"""Trainium2 Bass kernel for JonbertaSelfAttention (B=4,S=1024,DM=1024,H=16,D=64,SE=512,DF=512).

Single NEFF for all 8 cores: core c = (batch b = c//2) x (query-half qh = c%2).
The query-half offset l0 is folded into host-prepared inputs (hsqT slice and
shifted distance tables), so one compiled program serves both halves.

Layout: transposed scores S^T[r_part, l_free]; softmax denominators via a
ones-column appended to V; relative-position bias terms computed as banded
matmuls against shifted distance tables, written to DRAM scratch with a
skew-affine layout, and read back as [r, l] tiles -- the q-side via xbar
transpose-DMA (bf16), the k-side via a plain affine read (fp8) -- then
accumulated into the score PSUM with identity matmuls.
"""
import numpy as np
import ml_dtypes

BF16 = ml_dtypes.bfloat16
B, S, DM, H, D, SE, DF, MAXP = 4, 1024, 1024, 16, 64, 512, 512, 1024
L = 512          # query rows per core
NRT = S // 128   # 8 r-tiles (keys)
NLT = L // 128   # 4 l-tiles (local queries)
NET = SE // 128  # 4 encoder r-tiles
LN_EPS = 1e-12
QBW = 1151       # q-band width (d = r - l_local + 127 in [0, 1151))
KBW = 640        # k-band width (e = l - r_local + 127 in [0, 639])

_CACHE = {}
_JIT = {}


def _build():
    import concourse.bass as bass
    import concourse.mybir as mybir
    import concourse.tile as tile
    from concourse import bacc
    from concourse.masks import make_identity
    from contextlib import ExitStack

    dt = mybir.dt
    nc = bacc.Bacc("TRN2", target_bir_lowering=False, debug=False, num_devices=8)

    # ---- DRAM I/O ----
    # fp8 DoubleRow-packed activations and (x16-scaled) weights:
    # layout [128(k), nblk/2(p), 2(j), N] with row (2p+j)*128+k of the
    # transposed matrix in plane (p, j)
    d_hsq8 = nc.dram_tensor("hsq8", [128, NRT // 2, 2, L], dt.float8e4, kind="ExternalInput")
    d_hst8 = nc.dram_tensor("hst8", [128, NRT // 2, 2, S], dt.float8e4, kind="ExternalInput")
    d_enc8 = nc.dram_tensor("enc8", [128, NET // 2, 2, SE], dt.float8e4, kind="ExternalInput")
    d_wq8 = nc.dram_tensor("wq8", [128, NRT // 2, 2, DM], dt.float8e4, kind="ExternalInput")
    d_wk8 = nc.dram_tensor("wk8", [128, NRT // 2, 2, DM], dt.float8e4, kind="ExternalInput")
    d_wv8 = nc.dram_tensor("wv8", [128, NRT // 2, 2, DM], dt.float8e4, kind="ExternalInput")
    d_wfk8 = nc.dram_tensor("wfk8", [128, NET // 2, 2, DM], dt.float8e4, kind="ExternalInput")
    d_wfv8 = nc.dram_tensor("wfv8", [128, NET // 2, 2, DM], dt.float8e4, kind="ExternalInput")
    d_hsres = nc.dram_tensor("hsres", [L, DM], dt.float32, kind="ExternalInput")
    d_mask = nc.dram_tensor("mask", [S], dt.float32, kind="ExternalInput")
    d_woT = nc.dram_tensor("woT", [DM, DM], dt.bfloat16, kind="ExternalInput")
    d_bq = nc.dram_tensor("bq", [DM], dt.float32, kind="ExternalInput")
    d_bk = nc.dram_tensor("bk", [DM], dt.float32, kind="ExternalInput")
    d_bfk = nc.dram_tensor("bfk", [DM], dt.float32, kind="ExternalInput")
    d_bv = nc.dram_tensor("bv", [DM], dt.bfloat16, kind="ExternalInput")
    d_bfv = nc.dram_tensor("bfv", [DM], dt.bfloat16, kind="ExternalInput")
    d_bo = nc.dram_tensor("bo", [DM], dt.float32, kind="ExternalInput")
    d_lng = nc.dram_tensor("lng", [DM], dt.float32, kind="ExternalInput")
    d_lnb = nc.dram_tensor("lnb", [DM], dt.float32, kind="ExternalInput")
    # fq[:, y] = G[l0 + 1534 - y]; fk[:, z] = G[l0 + z]  (host-prepared, [64, 2048])
    d_fq = nc.dram_tensor("fq", [D, 2048], dt.bfloat16, kind="ExternalInput")
    d_fk = nc.dram_tensor("fk", [D, 2048], dt.bfloat16, kind="ExternalInput")
    d_out = nc.dram_tensor("out", [L, DM], dt.float32, kind="ExternalOutput")

    AP = bass.AP
    f32 = dt.float32
    bf16 = dt.bfloat16
    f8 = dt.float8e4
    f32r = dt.float32r
    AF = mybir.ActivationFunctionType

    with tile.TileContext(nc) as tc, ExitStack() as top:
        # ---------- DRAM scratch for bands ----------
        qbp = top.enter_context(tc.tile_pool(name="qbp", bufs=H, space="DRAM"))
        kbp = top.enter_context(tc.tile_pool(name="kbp", bufs=H, space="DRAM"))

        # ---------- persistent SBUF ----------
        pers = top.enter_context(tc.tile_pool(name="pers", bufs=1))
        kT = pers.tile([128, NRT, S], bf16, tag="kT")
        qT = pers.tile([128, NRT, L], bf16, tag="qT")
        fkT = pers.tile([128, NRT, L], bf16, tag="fkT")
        v_sb = pers.tile([128, NRT, H, 65], bf16, tag="v_sb")
        fv_sb = pers.tile([128, NET, H, 65], bf16, tag="fv_sb")
        ctxpk = pers.tile([128, NRT, L], bf16, tag="ctxpk")
        bv_b = pers.tile([128, DM], bf16, tag="bv_b")
        bfv_b = pers.tile([128, DM], bf16, tag="bfv_b")
        msk = pers.tile([128, NRT], f32, tag="msk")
        msk8 = pers.tile([128, NRT], f32, tag="msk8")
        ident = pers.tile([128, 128], bf16, tag="ident")
        identf8 = pers.tile([128, 128], f8, tag="identf8")
        ones65 = pers.tile([65, D], bf16, tag="ones65")
        eps_t = pers.tile([128, 1], f32, tag="eps_t")
        zero_t = pers.tile([128, 1], f32, tag="zero_t")

        make_identity(nc, ident[:])
        nc.gpsimd.tensor_copy(identf8[:], ident[:])
        nc.gpsimd.memset(ones65[:], 1.0)
        nc.vector.memset(eps_t[:], LN_EPS)
        nc.vector.memset(zero_t[:], 0.0)
        nc.gpsimd.dma_start(out=bv_b[:], in_=AP(tensor=d_bv, offset=0, ap=[[0, 128], [1, DM]]))
        nc.gpsimd.dma_start(out=bfv_b[:], in_=AP(tensor=d_bfv, offset=0, ap=[[0, 128], [1, DM]]))
        nc.gpsimd.dma_start(out=msk[:], in_=AP(tensor=d_mask, offset=0, ap=[[1, 128], [128, NRT]]))
        nc.gpsimd.tensor_scalar_mul(out=msk8[:], in0=msk[:], scalar1=8.0)
        nc.vector.memset(v_sb[:, :, :, 64:65], 1.0)
        nc.vector.memset(fv_sb[:, :, :, 64:65], 1.0)

        qb_dram = {}
        kb_dram = {}

        # band-extraction destination pools + read issuer (reads for the first
        # heads are issued inside phase B as soon as their bands hit DRAM)
        b1p = top.enter_context(tc.tile_pool(name="b1p", bufs=4 * NRT))
        b2p = top.enter_context(tc.tile_pool(name="b2p", bufs=4))
        b1t = {}
        b2t = {}

        def issue_read_b1(h, rt):
            qb = qb_dram[h]
            t = b1p.tile([128, L], bf16, tag="b1")
            nc.sync.dma_start(
                out=t[:],
                in_=AP(tensor=qb.tensor, offset=qb.offset + 127 + 128 * rt,
                       ap=[[QBW, L], [1, 128]]),
                transpose=True)
            b1t[(h, rt)] = t

        def issue_read_b2(h):
            kb = kb_dram[h]
            t = b2p.tile([128, NRT, L], f8, tag="b2")
            nc.sync.dma_start(
                out=t[:],
                in_=AP(tensor=kb.tensor, offset=kb.offset + 127,
                       ap=[[KBW - 1, 128], [128 * KBW, NRT], [1, L]]))
            b2t[h] = t

        def issue_reads(h):
            qb = qb_dram[h]
            # q-side: xbar transpose read of [l, r]-diag -> [r, l] tiles,
            # split across both HWDGE rings
            for rt in range(NRT):
                t = b1p.tile([128, L], bf16, tag="b1")
                nc.sync.dma_start(
                    out=t[:],
                    in_=AP(tensor=qb.tensor, offset=qb.offset + 127 + 128 * rt,
                           ap=[[QBW, L], [1, 128]]),
                    transpose=True)
                b1t[(h, rt)] = t
            kb = kb_dram[h]
            t = b2p.tile([128, NRT, L], f8, tag="b2")
            nc.sync.dma_start(
                out=t[:],
                in_=AP(tensor=kb.tensor, offset=kb.offset + 127,
                       ap=[[KBW - 1, 128], [128 * KBW, NRT], [1, L]]))
            b2t[h] = t

        # ================= phase B: projections + bands + V/FV =================
        with ExitStack() as phB:
            pb = phB.enter_context(tc.tile_pool(name="pb", bufs=1))
            hst8 = pb.tile([128, NRT // 2, 2, S], f8, tag="hst8")
            hsq8 = pb.tile([128, NRT // 2, 2, L], f8, tag="hsq8")
            enc8 = pb.tile([128, NET // 2, 2, SE], f8, tag="enc8")
            fq_s = pb.tile([128, 2048], bf16, tag="fq")
            fk_s = pb.tile([128, 2048], bf16, tag="fk")
            bq_s = pb.tile([128, NRT], f32, tag="bq_s")
            bk_s = pb.tile([128, NRT], f32, tag="bk_s")
            bfk_s = pb.tile([128, NRT], f32, tag="bfk_s")

            # ---- Q/K/FK projections ----
            with ExitStack() as w1:
                wp = w1.enter_context(tc.tile_pool(name="wp", bufs=1))
                wq8 = wp.tile([128, NRT // 2, 2, DM], f8, tag="wq8")
                wk8 = wp.tile([128, NRT // 2, 2, DM], f8, tag="wk8")
                wfk8 = wp.tile([128, NET // 2, 2, DM], f8, tag="wfk8")
                pp1 = w1.enter_context(tc.tile_pool(name="pp1", bufs=4, space="PSUM"))

                # load order = first-use order
                nc.sync.dma_start(out=wq8[:, 0:2], in_=d_wq8[:, 0:2, :, :])
                nc.sync.dma_start(out=hsq8[:], in_=d_hsq8[:, :, :, :])
                nc.sync.dma_start(out=wq8[:, 2:4], in_=d_wq8[:, 2:4, :, :])
                nc.sync.dma_start(out=wk8[:, 0:2], in_=d_wk8[:, 0:2, :, :])
                nc.sync.dma_start(out=hst8[:, 0:2], in_=d_hst8[:, 0:2, :, :])
                nc.sync.dma_start(out=wk8[:, 2:4], in_=d_wk8[:, 2:4, :, :])
                nc.sync.dma_start(out=hst8[:, 2:4], in_=d_hst8[:, 2:4, :, :])
                nc.sync.dma_start(out=wfk8[:], in_=d_wfk8[:, :, :, :])
                nc.sync.dma_start(out=enc8[:], in_=d_enc8[:, :, :, :])
                nc.gpsimd.dma_start(out=bq_s[:], in_=AP(tensor=d_bq, offset=0, ap=[[1, 128], [128, NRT]]))
                nc.gpsimd.dma_start(out=bk_s[:], in_=AP(tensor=d_bk, offset=0, ap=[[1, 128], [128, NRT]]))
                nc.gpsimd.dma_start(out=bfk_s[:], in_=AP(tensor=d_bfk, offset=0, ap=[[1, 128], [128, NRT]]))
                for half in range(2):
                    nc.scalar.dma_start(out=fq_s[half * 64:(half + 1) * 64, :],
                                        in_=AP(tensor=d_fq, offset=0, ap=[[2048, 64], [1, 2048]]))
                    nc.scalar.dma_start(out=fk_s[half * 64:(half + 1) * 64, :],
                                        in_=AP(tensor=d_fk, offset=0, ap=[[2048, 64], [1, 2048]]))

                DR = mybir.MatmulPerfMode.DoubleRow
                for ot in range(NRT):
                    ps = pp1.tile([128, 512], f32, tag="pp1")
                    for p in range(NRT // 2):
                        nc.tensor.matmul(ps[:], lhsT=wq8[:, p, :, ot * 128:(ot + 1) * 128],
                                         rhs=hsq8[:, p, :, :], perf_mode=DR,
                                         start=(p == 0), stop=(p == NRT // 2 - 1))
                    nc.scalar.activation(out=qT[:, ot, :], in_=ps[:], func=AF.Identity,
                                         bias=bq_s[:, ot:ot + 1], scale=1.0 / 16)
                    for sb_i in range(2):
                        ps = pp1.tile([128, 512], f32, tag="pp1")
                        for p in range(NRT // 2):
                            nc.tensor.matmul(ps[:], lhsT=wk8[:, p, :, ot * 128:(ot + 1) * 128],
                                             rhs=hst8[:, p, :, sb_i * 512:(sb_i + 1) * 512],
                                             perf_mode=DR,
                                             start=(p == 0), stop=(p == NRT // 2 - 1))
                        nc.scalar.activation(out=kT[:, ot, sb_i * 512:(sb_i + 1) * 512],
                                             in_=ps[:], func=AF.Identity,
                                             bias=bk_s[:, ot:ot + 1], scale=1.0 / 16)
                    ps = pp1.tile([128, 512], f32, tag="pp1")
                    for p in range(NET // 2):
                        nc.tensor.matmul(ps[:], lhsT=wfk8[:, p, :, ot * 128:(ot + 1) * 128],
                                         rhs=enc8[:, p, :, :], perf_mode=DR,
                                         start=(p == 0), stop=(p == NET // 2 - 1))
                    nc.scalar.activation(out=fkT[:, ot, :], in_=ps[:], func=AF.Identity,
                                         bias=bfk_s[:, ot:ot + 1], scale=1.0 / 16)

            # ---- bands + V/FV ----
            with ExitStack() as w2:
                wp2 = w2.enter_context(tc.tile_pool(name="wp2", bufs=1))
                wv8 = wp2.tile([128, NRT // 2, 2, DM], f8, tag="wv8")
                wfv8 = wp2.tile([128, NET // 2, 2, DM], f8, tag="wfv8")
                # band PSUM: 12 "A" tiles/head [128,640] (2 banks, 3-deep) and
                # 4 q-tail tiles/head [128,511] sharing the 1-bank pool with
                # the V/FV projections -> exactly 8 banks
                ppb = w2.enter_context(tc.tile_pool(name="ppb", bufs=3, space="PSUM"))
                ppv = w2.enter_context(tc.tile_pool(name="ppv", bufs=2, space="PSUM"))
                qsp = w2.enter_context(tc.tile_pool(name="qsp", bufs=3))
                ksp = w2.enter_context(tc.tile_pool(name="ksp", bufs=3))

                nc.sync.dma_start(out=wv8[:], in_=d_wv8[:, :, :, :])
                nc.sync.dma_start(out=wfv8[:], in_=d_wfv8[:, :, :, :])
                DR2 = mybir.MatmulPerfMode.DoubleRow

                def v_block(st, ob):
                    ps = ppv.tile([128, 512], f32, tag="small")
                    for p in range(NRT // 2):
                        nc.tensor.matmul(ps[:], lhsT=hst8[:, p, :, st * 128:(st + 1) * 128],
                                         rhs=wv8[:, p, :, ob * 512:(ob + 1) * 512],
                                         perf_mode=DR2,
                                         start=(p == 0), stop=(p == NRT // 2 - 1))
                    nc.vector.scalar_tensor_tensor(
                        out=v_sb[:, st, ob * 8:(ob + 1) * 8, 0:64],
                        in0=ps[:].rearrange("p (h d) -> p h d", d=64),
                        scalar=1.0 / 16, op0=mybir.AluOpType.mult, op1=mybir.AluOpType.add,
                        in1=bv_b[:, ob * 512:(ob + 1) * 512].rearrange("p (h d) -> p h d", d=64))

                def fv_block(st, ob):
                    ps = ppv.tile([128, 512], f32, tag="small")
                    for p in range(NET // 2):
                        nc.tensor.matmul(ps[:], lhsT=enc8[:, p, :, st * 128:(st + 1) * 128],
                                         rhs=wfv8[:, p, :, ob * 512:(ob + 1) * 512],
                                         perf_mode=DR2,
                                         start=(p == 0), stop=(p == NET // 2 - 1))
                    nc.vector.scalar_tensor_tensor(
                        out=fv_sb[:, st, ob * 8:(ob + 1) * 8, 0:64],
                        in0=ps[:].rearrange("p (h d) -> p h d", d=64),
                        scalar=1.0 / 16, op0=mybir.AluOpType.mult, op1=mybir.AluOpType.add,
                        in1=bfv_b[:, ob * 512:(ob + 1) * 512].rearrange("p (h d) -> p h d", d=64))

                vblocks = [(v_block, st, ob) for st in range(NRT) for ob in range(2)] + \
                          [(fv_block, st, ob) for st in range(NET) for ob in range(2)]
                rq = []
                post_band = []

                for h in range(H):
                    hp = (h % 2) * 64
                    ot = h // 2
                    # q-band: rows l_local, cols d = r - l_local + 127
                    qs = qsp.tile([128, NLT, 1152], bf16, tag="qs")
                    kss = ksp.tile([128, NRT, KBW], f8, tag="ks")
                    for lt in range(NLT):
                        base = 384 - 128 * lt
                        psa = ppb.tile([128, KBW], f32, tag="band")
                        for n0, nn in ((0, 512), (512, 128)):
                            nc.tensor.matmul(psa[:, n0:n0 + nn],
                                             lhsT=qT[hp:hp + 64, ot, lt * 128:(lt + 1) * 128],
                                             rhs=fq_s[hp:hp + 64, base + n0:base + n0 + nn],
                                             start=True, stop=True, skip_group_check=True)
                        psb = ppv.tile([128, 512], f32, tag="small")
                        nc.tensor.matmul(psb[:, 0:QBW - 640],
                                         lhsT=qT[hp:hp + 64, ot, lt * 128:(lt + 1) * 128],
                                         rhs=fq_s[hp:hp + 64, base + 640:base + QBW],
                                         start=True, stop=True, skip_group_check=True)
                        if lt % 2 == 0:
                            nc.scalar.copy(out=qs[:, lt, 0:640], in_=psa[:])
                            nc.vector.tensor_copy(qs[:, lt, 640:QBW], psb[:, 0:QBW - 640])
                        else:
                            nc.vector.tensor_copy(qs[:, lt, 0:640], psa[:])
                            nc.scalar.copy(out=qs[:, lt, 640:QBW], in_=psb[:, 0:QBW - 640])
                    for rt in range(NRT):
                        base = 896 - 128 * rt
                        ps = ppb.tile([128, KBW], f32, tag="band")
                        for n0, nn in ((0, 512), (512, 128)):
                            nc.tensor.matmul(ps[:, n0:n0 + nn],
                                             lhsT=kT[hp:hp + 64, ot, rt * 128:(rt + 1) * 128],
                                             rhs=fk_s[hp:hp + 64, base + n0:base + n0 + nn],
                                             start=True, stop=True, skip_group_check=True)
                        if rt in (0, 2, 4, 5, 7):
                            nc.scalar.copy(out=kss[:, rt, :], in_=ps[:])
                        else:
                            nc.vector.tensor_copy(kss[:, rt, :], ps[:])
                    qb = qbp.tile([NLT, 128, 1152], bf16, tag="qb")
                    qb_dram[h] = qb
                    # skew-affine store: addr(l, r) = 1151*l + r + 127
                    nc.sync.dma_start(
                        out=AP(tensor=qb.tensor, offset=qb.offset,
                               ap=[[1152, 128], [1151 * 128, NLT], [1, QBW]]),
                        in_=qs[:, :, 0:QBW])
                    kb = kbp.tile([NRT, 128, KBW], f8, tag="kb")
                    kb_dram[h] = kb
                    nc.sync.dma_start(
                        out=AP(tensor=kb.tensor, offset=kb.offset,
                               ap=[[KBW, 128], [128 * KBW, NRT], [1, KBW]]),
                        in_=kss[:])
                    # interleave V/FV projection blocks to keep PE fed while
                    # Act/DVE chew on the band copies
                    lo, hi = (24 * h) // H, (24 * (h + 1)) // H
                    for fn, st, ob in vblocks[lo:hi]:
                        fn(st, ob)
                    if h < 4:
                        issue_reads(h)

        # phase-D tensors: loaded (via SWDGE, off the critical rings) during
        # phase C so phase D starts immediately
        pd = top.enter_context(tc.tile_pool(name="pd", bufs=1))
        wo_s = pd.tile([128, NRT, DM], bf16, tag="wo")
        hsr_s = pd.tile([128, NLT, DM], f32, tag="hsr")
        bo_b = pd.tile([128, DM], f32, tag="bo_b")
        lng_b = pd.tile([128, DM], f32, tag="lng_b")
        lnb_b = pd.tile([128, DM], f32, tag="lnb_b")

        def issue_d_loads():
            nc.gpsimd.dma_start(out=wo_s[:], in_=AP(tensor=d_woT, offset=0,
                                ap=[[DM, 128], [128 * DM, NRT], [1, DM]]))
            nc.gpsimd.dma_start(out=hsr_s[:], in_=AP(tensor=d_hsres, offset=0,
                                ap=[[DM, 128], [128 * DM, NLT], [1, DM]]))
            nc.gpsimd.dma_start(out=bo_b[:], in_=AP(tensor=d_bo, offset=0, ap=[[0, 128], [1, DM]]))
            nc.gpsimd.dma_start(out=lng_b[:], in_=AP(tensor=d_lng, offset=0, ap=[[0, 128], [1, DM]]))
            nc.gpsimd.dma_start(out=lnb_b[:], in_=AP(tensor=d_lnb, offset=0, ap=[[0, 128], [1, DM]]))

        # ================= phase C: attention =================
        with ExitStack() as phC:
            bsp = phC.enter_context(tc.tile_pool(name="bsp", bufs=10))
            exp_p = phC.enter_context(tc.tile_pool(name="exp_p", bufs=4))
            cmb = phC.enter_context(tc.tile_pool(name="cmb", bufs=4))
            rcp = phC.enter_context(tc.tile_pool(name="rcp", bufs=4))
            pps = phC.enter_context(tc.tile_pool(name="pps", bufs=3, space="PSUM"))
            ppc = phC.enter_context(tc.tile_pool(name="ppc", bufs=4, space="PSUM"))
            ppr = phC.enter_context(tc.tile_pool(name="ppr", bufs=1, space="PSUM"))

            def combine(h, ctx, ctxe):
                # normalize + combine (PE broadcast of reciprocal rows);
                # emitted a few score-groups into the NEXT head so the PE
                # never waits on the recip chain
                hp = (h % 2) * 64
                ot = h // 2
                recb = rcp.tile([65, 512], bf16, tag="recb")
                recbe = rcp.tile([65, 512], bf16, tag="recb")
                with nc.allow_low_precision(reason="softmax denom recip in bf16"):
                    nc.vector.reciprocal(out=recb[64:65, :], in_=ctx[64:65, :])
                    nc.vector.reciprocal(out=recbe[64:65, :], in_=ctxe[64:65, :])
                rb = ppr.tile([64, 512], f32, tag="rb")
                rbe = ppr.tile([64, 512], f32, tag="rb")
                nc.tensor.matmul(rb[:], lhsT=ones65[64:65, :],
                                 rhs=recb[64:65, :], start=True, stop=True)
                nc.tensor.matmul(rbe[:], lhsT=ones65[64:65, :],
                                 rhs=recbe[64:65, :], start=True, stop=True)
                rbs = cmb.tile([64, 512], bf16, tag="rbs")
                rbes = cmb.tile([64, 512], bf16, tag="rbs")
                nc.scalar.copy(out=rbs[:], in_=rb[:])
                nc.scalar.copy(out=rbes[:], in_=rbe[:])
                t1 = cmb.tile([64, 512], f32, tag="t1")
                t2 = cmb.tile([64, 512], f32, tag="t2")
                nc.vector.tensor_mul(out=t1[:], in0=ctx[0:64, :], in1=rbs[:])
                nc.vector.tensor_mul(out=t2[:], in0=ctxe[0:64, :], in1=rbes[:])
                nc.vector.tensor_add(out=ctxpk[hp:hp + 64, ot, :], in0=t1[:], in1=t2[:])

            pending = None
            for h in range(H):
                hp = (h % 2) * 64
                ot = h // 2
                if h == 2:
                    issue_d_loads()
                ctx = ppc.tile([65, 512], f32, tag="ctx")
                ctxe = ppc.tile([65, 512], f32, tag="ctx")

                def pre_add(rt):
                    # pre-sum the two bias tiles on DVE so the PE pays only
                    # one identity accumulation per score tile
                    bsum = bsp.tile([128, 512], bf16, tag="bsum")
                    nc.vector.tensor_add(out=bsum[:], in0=b1t.pop((h, rt))[:],
                                         in1=b2t[h][:, rt, :])
                    bsums[rt] = bsum

                bsums = {}
                pre_add(0)
                pre_add(1)
                exs = []
                for rt in range(NRT + 1):
                    if rt < NRT:
                        if rt + 2 < NRT:
                            pre_add(rt + 2)
                        ps = pps.tile([128, 512], f32, tag="sc")
                        nc.tensor.matmul(ps[:], lhsT=kT[hp:hp + 64, ot, rt * 128:(rt + 1) * 128],
                                         rhs=qT[hp:hp + 64, ot, :],
                                         start=True, stop=False, skip_group_check=True)
                        nc.tensor.matmul(ps[:], lhsT=ident[:], rhs=bsums.pop(rt)[:],
                                         start=False, stop=True, skip_group_check=True)
                        ex = exp_p.tile([128, 512], bf16, tag="ex")
                        nc.scalar.activation(out=ex[:], in_=ps[:], func=AF.Exp,
                                             bias=msk[:, rt:rt + 1], scale=0.125)
                        exs.append(ex)
                    if rt == 2 and pending is not None:
                        combine(*pending)
                        pending = None
                    if h + 4 < H and rt < NRT:
                        issue_read_b1(h + 4, rt)
                        if rt == 0:
                            issue_read_b2(h + 4)
                    if rt > 0:
                        nc.tensor.matmul(ctx[:], lhsT=v_sb[:, rt - 1, h, :], rhs=exs[rt - 1][:],
                                         start=(rt == 1), stop=(rt == NRT),
                                         skip_group_check=True)
                exs = []
                for ret in range(NET + 1):
                    if ret < NET:
                        ps = pps.tile([128, 512], f32, tag="sc")
                        nc.tensor.matmul(ps[:], lhsT=fkT[hp:hp + 64, ot, ret * 128:(ret + 1) * 128],
                                         rhs=qT[hp:hp + 64, ot, :], start=True, stop=True)
                        ex = exp_p.tile([128, 512], bf16, tag="ex")
                        nc.scalar.activation(out=ex[:], in_=ps[:], func=AF.Exp,
                                             bias=zero_t[:], scale=0.125)
                        exs.append(ex)
                    if ret > 0:
                        nc.tensor.matmul(ctxe[:], lhsT=fv_sb[:, ret - 1, h, :], rhs=exs[ret - 1][:],
                                         start=(ret == 1), stop=(ret == NET),
                                         skip_group_check=True)
                pending = (h, ctx, ctxe)
            combine(*pending)

        # ================= phase D: output dense + residual + LN =================
        with ExitStack() as phD:
            yp = phD.enter_context(tc.tile_pool(name="yp", bufs=2))
            op = phD.enter_context(tc.tile_pool(name="op", bufs=2))
            stp = phD.enter_context(tc.tile_pool(name="stp", bufs=2))
            ppy = phD.enter_context(tc.tile_pool(name="ppy", bufs=2, space="PSUM"))

            for st in range(NLT):
                nc.gpsimd.tensor_add(out=hsr_s[:, st, :], in0=hsr_s[:, st, :], in1=bo_b[:])
            for st in range(NLT):
                y = yp.tile([128, DM], f32, tag="y")
                for ob in range(2):
                    ps = ppy.tile([128, 512], f32, tag="py")
                    for ct in range(NRT):
                        nc.tensor.matmul(ps[:], lhsT=ctxpk[:, ct, st * 128:(st + 1) * 128],
                                         rhs=wo_s[:, ct, ob * 512:(ob + 1) * 512],
                                         start=(ct == 0), stop=(ct == NRT - 1))
                    nc.vector.tensor_add(out=y[:, ob * 512:(ob + 1) * 512], in0=ps[:],
                                         in1=hsr_s[:, st, ob * 512:(ob + 1) * 512])
                stats = stp.tile([128, 2, 6], f32, tag="stats")
                nc.vector.bn_stats(out=stats[:, 0, :], in_=y[:, 0:512])
                nc.vector.bn_stats(out=stats[:, 1, :], in_=y[:, 512:1024])
                mv = stp.tile([128, 2], f32, tag="mv")
                nc.vector.bn_aggr(out=mv[:], in_=stats[:])
                sd = stp.tile([128, 1], f32, tag="sd")
                nc.scalar.activation(out=sd[:], in_=mv[:, 1:2], func=AF.Sqrt,
                                     bias=eps_t[:], scale=1.0)
                rstd = stp.tile([128, 1], f32, tag="rstd")
                nc.vector.reciprocal(out=rstd[:], in_=sd[:])
                eng = nc.gpsimd if st % 2 == 0 else nc.vector
                o1 = op.tile([128, DM], f32, tag="o1")
                eng.tensor_scalar(out=o1[:], in0=y[:], scalar1=mv[:, 0:1], scalar2=rstd[:],
                                  op0=mybir.AluOpType.subtract, op1=mybir.AluOpType.mult)
                o2 = op.tile([128, DM], f32, tag="o2")
                eng.tensor_mul(out=o2[:], in0=o1[:], in1=lng_b[:])
                o3 = op.tile([128, DM], f32, tag="o3")
                eng.tensor_add(out=o3[:], in0=o2[:], in1=lnb_b[:])
                nc.sync.dma_start(out=d_out[st * 128:(st + 1) * 128, :], in_=o3[:])

    nc.finalize()
    return nc


def _get_nc():
    if "nc" not in _CACHE:
        _CACHE["nc"] = _build()
    return _CACHE["nc"]


def _get_runner(nc, n_cores=8):
    """Build (once) a cached jitted SPMD callable for the bass module."""
    key = id(nc)
    if key in _JIT:
        return _JIT[key]
    import jax
    import concourse.mybir as mybir
    from concourse import bass2jax
    from jax.experimental.shard_map import shard_map
    from jax.sharding import Mesh, PartitionSpec

    bass2jax.install_neuronx_cc_hook()

    in_names, out_names, out_avals, zero_outs = [], [], [], []
    partition_name = nc.partition_id_tensor.name if nc.partition_id_tensor else None
    for alloc in nc.m.functions[0].allocations:
        if not isinstance(alloc, mybir.MemoryLocationSet):
            continue
        name = alloc.memorylocations[0].name
        if alloc.kind == "ExternalInput":
            if name != partition_name:
                in_names.append(name)
        elif alloc.kind == "ExternalOutput":
            out_names.append(name)
            shape = tuple(alloc.tensor_shape)
            dtype = mybir.dt.np(alloc.dtype)
            out_avals.append(jax.core.ShapedArray(shape, dtype))
            zero_outs.append(np.zeros(shape, dtype))
    n_params = len(in_names)
    n_outs = len(out_avals)
    all_names = list(in_names) + list(out_names)
    if partition_name is not None:
        all_names.append(partition_name)
    donate = tuple(range(n_params, n_params + n_outs))

    def _body(*args):
        operands = list(args)
        if partition_name is not None:
            operands.append(bass2jax.partition_id_tensor())
        outs = bass2jax._bass_exec_p.bind(
            *operands,
            out_avals=tuple(out_avals),
            in_names=tuple(all_names),
            out_names=tuple(out_names),
            lowering_input_output_aliases=(),
            sim_require_finite=True,
            sim_require_nnan=True,
            nc=nc,
        )
        return tuple(outs)

    devices = jax.devices()[:n_cores]
    assert len(devices) == n_cores
    mesh = Mesh(np.asarray(devices), ("core",))
    in_specs = (PartitionSpec("core"),) * (n_params + n_outs)
    out_specs = (PartitionSpec("core"),) * n_outs
    sharded = jax.jit(
        shard_map(_body, mesh=mesh, in_specs=in_specs, out_specs=out_specs,
                  check_rep=False),
        keep_unused=True)

    def prep(in_maps):
        from jax.sharding import NamedSharding
        sh = NamedSharding(mesh, PartitionSpec("core"))
        per_core = [[np.asarray(m[name]) for name in in_names] for m in in_maps]
        concat_in = [np.concatenate([per_core[c][i] for c in range(n_cores)], axis=0)
                     for i in range(n_params)]
        concat_zeros = [np.zeros((n_cores * z.shape[0], *z.shape[1:]), z.dtype)
                        for z in zero_outs]
        return [jax.device_put(a, sh) for a in concat_in + concat_zeros]

    def run(dev_args):
        out_arrs = sharded(*dev_args)
        jax.block_until_ready(out_arrs)
        return out_arrs

    def fetch(out_arrs):
        out_np = [np.asarray(a) for a in out_arrs]
        return [
            {name: out_np[i].reshape(n_cores, *out_avals[i].shape)[c]
             for i, name in enumerate(out_names)}
            for c in range(n_cores)
        ]

    _JIT[key] = (prep, run, fetch)
    return _JIT[key]


LAST_EXEC_NS = None

F8 = ml_dtypes.float8_e4m3


def _drpack(xT, scale=1.0):
    """[K, N] -> fp8 DoubleRow pack [128, K/256, 2, N]."""
    Kd, N = xT.shape
    nb = Kd // 128
    x = np.clip(xT.astype(np.float32) * scale, -240, 240).astype(F8)
    return np.ascontiguousarray(
        x.reshape(nb // 2, 2, 128, N).transpose(2, 0, 1, 3))


def make_in_maps(inputs):
    inp = {k: np.asarray(v) for k, v in inputs.items()}
    hs = inp["hidden_states"].astype(np.float32)
    mask = inp["attention_mask"].astype(np.float32)
    enc = inp["encoder_hidden_states"].astype(np.float32)
    G = inp["dist_emb"].astype(np.float32)

    def b16(x):
        return np.ascontiguousarray(x.astype(BF16))

    shared = {
        "wq8": _drpack(inp["Wq"].T, 16.0), "wk8": _drpack(inp["Wk"].T, 16.0),
        "wv8": _drpack(inp["Wv"].T, 16.0), "wfk8": _drpack(inp["Wfk"].T, 16.0),
        "wfv8": _drpack(inp["Wfv"].T, 16.0), "woT": b16(inp["Wo"].T),
        "bq": inp["bq"].astype(np.float32), "bk": inp["bk"].astype(np.float32),
        "bfk": inp["bfk"].astype(np.float32), "bv": b16(inp["bv"]), "bfv": b16(inp["bfv"]),
        "bo": inp["bo"].astype(np.float32), "lng": inp["ln_g"].astype(np.float32),
        "lnb": inp["ln_b"].astype(np.float32),
    }
    # per-query-half shifted distance tables
    # fq[:, y] = G[l0 + 1534 - y], fk[:, z] = G[l0 + z]
    fq_h, fk_h = {}, {}
    for qh in (0, 1):
        l0 = qh * L
        fq = np.zeros((2048, D), np.float32)
        idx = l0 + 1534 - np.arange(2048)
        val = (idx >= 0) & (idx < 2 * MAXP - 1)
        fq[val] = G[idx[val]]
        fk = np.zeros((2048, D), np.float32)
        idx = l0 + np.arange(2048)
        val = idx < 2 * MAXP - 1
        fk[val] = G[idx[val]]
        fq_h[qh] = b16(fq.T)
        fk_h[qh] = b16(fk.T)

    in_maps = []
    for c in range(8):
        b, qh = c // 2, c % 2
        l0 = qh * L
        m = dict(shared)
        m["hst8"] = _drpack(hs[b].T)
        m["hsq8"] = _drpack(hs[b, l0:l0 + L, :].T)
        m["enc8"] = _drpack(enc[b].T)
        m["hsres"] = np.ascontiguousarray(hs[b, l0:l0 + L, :])
        m["mask"] = np.ascontiguousarray(np.broadcast_to(mask[b, 0, 0, :], (S,)))
        m["fq"] = fq_h[qh]
        m["fk"] = fk_h[qh]
        in_maps.append(m)
    return in_maps


def kernel(**inputs):
    global LAST_EXEC_NS
    import time

    in_maps = make_in_maps(inputs)
    nc = _get_nc()
    prep, run, fetch = _get_runner(nc, 8)
    dev_args = prep(in_maps)
    _CACHE["dev_args"] = dev_args
    out_arrs = run(dev_args)  # warm-up / compile
    times = []
    for _ in range(4):
        t0 = time.perf_counter()
        out_arrs = run(dev_args)
        times.append(time.perf_counter() - t0)
    LAST_EXEC_NS = int(min(times) * 1e9)
    res = fetch(out_arrs)

    out = np.zeros((B, S, DM), np.float32)
    for c in range(8):
        b, qh = c // 2, c % 2
        out[b, qh * L:(qh + 1) * L, :] = res[c]["out"]
    return out
